# revision 24
# baseline (speedup 1.0000x reference)
"""HGNN layer kernel for 8 TRN2 NeuronCores (Bass/Tile, SPMD row-sharded).

Math (reference):
    dv = H.sum(1); de = H.sum(0)
    Xs = X * dv^-1/2
    M  = H^T @ Xs            [E, F]
    M  = M * de^-1
    Xn = (H @ M) * dv^-1/2   [N, F]
    out = Xn @ W^T + b

Distribution: rows of X/H sharded over 8 cores (N=8192 -> 1024 rows/core).
GEMM1 (H^T @ Xs) is a local partial GEMM; the [E, F] partial plus the
partial column-sum row `de` are fused into ONE AllReduce of [F+1, E] in
bf16 (H is 0/1 so bf16 is exact for it; everything accumulates in f32
PSUM). Everything after the AllReduce is row-parallel.

Scheduling notes (each fixes a measured stall):
  * All large operands are bf16: halves HBM traffic + AllReduce payload,
    and bf16 matmuls stream at the full 2.4 GHz rate (f32r capped lower).
  * dv row-sums run on the Scalar engine (activation Copy + accum_out);
    de partials run on the PE as ones-stationary matmuls interleaved
    with GEMM1. This keeps the DVE free for the GEMM1 PSUM->bf16 casts,
    which gate the collective trigger.
  * The de row of the collective buffer is written/read CONTIGUOUSLY
    ([1,E] row write; [8,128] read + PE transpose for per-partition
    scalars). The earlier (c p) scatter view cost ~12 us of 2-byte
    descriptors and delayed the AllReduce trigger by that much.
  * H^T loads are issued on the gpsimd queue AFTER the collective
    trigger so their descriptors don't delay it; they overlap the
    AllReduce and are only needed by GEMM2.
  * GEMM1 runs e-half outer / n-tile inner with 2 open PSUM banks, so
    matmuls start as soon as h[0]/xs[0] land.
"""

import os
import sys
import types

import numpy as np
import ml_dtypes


def _ensure_axon_hooks_module():
    """bass_utils imports antenv.axon_hooks when tracing; some images
    lack it. Provide a stub (and try to wire the real ctypes hook) so
    trace paths degrade gracefully instead of crashing."""
    try:
        import antenv.axon_hooks  # noqa: F401
        return
    except ImportError:
        pass
    try:
        import antenv
    except ImportError:
        return
    mod = types.ModuleType("antenv.axon_hooks")
    state = {"hook": None}
    mod.get_axon_ntff_profile_hook = lambda: state["hook"]
    mod.set_axon_ntff_profile_hook = lambda h: state.__setitem__("hook", h)
    sys.modules["antenv.axon_hooks"] = mod
    antenv.axon_hooks = mod
    try:
        from trn_agent_boot.trn_boot import _ntff_profile_via_ctypes
        hook = _ntff_profile_via_ctypes("/opt/axon/libaxon_pjrt.so")
        if hook is not None:
            state["hook"] = hook
    except Exception:
        pass


_ensure_axon_hooks_module()

N, E, F = 8192, 1024, 256
P = 128
NC_COUNT = 8
NL = N // NC_COUNT          # 1024 rows per core
NT = NL // P                # 8 row tiles per core
ET = E // P                 # 8 e-chunks
FI = F // P                 # 2 fi-chunks
EH = 512                    # moving free-dim per GEMM1 matmul / e-half

_cache = {}


def _build():
    from concourse import bacc, bass, tile, mybir

    f32 = mybir.dt.float32
    bf16 = mybir.dt.bfloat16

    nc = bacc.Bacc("TRN2", target_bir_lowering=False, debug=False,
                   num_devices=NC_COUNT)

    X_d = nc.dram_tensor("X", [NL, F], bf16, kind="ExternalInput")
    H_d = nc.dram_tensor("H", [NL, E], bf16, kind="ExternalInput")
    HT_d = nc.dram_tensor("HT", [E, NL], bf16, kind="ExternalInput")
    WT_d = nc.dram_tensor("WT", [F, F], bf16, kind="ExternalInput")
    B_d = nc.dram_tensor("bias", [P, F], f32, kind="ExternalInput")
    ONES_d = nc.dram_tensor("ones", [P, 1], bf16, kind="ExternalInput")
    ID8_d = nc.dram_tensor("ident8", [ET, ET], bf16, kind="ExternalInput")
    out_d = nc.dram_tensor("out", [NL, F], f32, kind="ExternalOutput")

    with tile.TileContext(nc) as tc:
        with (
            tc.tile_pool(name="const", bufs=1) as constp,
            tc.tile_pool(name="hp", bufs=1) as hp,
            tc.tile_pool(name="htp", bufs=1) as htp,
            tc.tile_pool(name="xp", bufs=1) as xp,
            tc.tile_pool(name="sp", bufs=1) as sp,
            tc.tile_pool(name="mtout", bufs=4) as mtoutp,
            tc.tile_pool(name="mwp", bufs=1) as mwp,
            tc.tile_pool(name="outp", bufs=4) as outp,
            tc.tile_pool(name="ps_mt", bufs=1, space="PSUM") as ps_mt,
            tc.tile_pool(name="ps_de", bufs=1, space="PSUM") as ps_de,
            tc.tile_pool(name="ps_w", bufs=2, space="PSUM") as ps_w,
            tc.tile_pool(name="ps_o", bufs=2, space="PSUM") as ps_o,
            tc.tile_pool(name="dram", bufs=1, space="DRAM") as dramp,
        ):
            # ---- input loads. H on the sync queue (gates dv and GEMM1);
            # ones FIRST on the gpsimd queue so the de matmuls (ones
            # stationary) can start as soon as h0 lands — putting ones
            # behind the H loads on the sync queue made the de matmuls
            # wait on the queue's cumulative semaphore for ALL H tiles,
            # idling the PE until ~23us. X then weights follow on gpsimd.
            h = []
            for i in range(NT):
                hi = hp.tile([P, E], bf16, name=f"h{i}")
                nc.sync.dma_start(hi[:], H_d[i * P:(i + 1) * P, :])
                h.append(hi)
            ones = constp.tile([P, 1], bf16)
            nc.gpsimd.dma_start(ones[:], ONES_d[:, :])

            x = []
            for i in range(NT):
                xi = xp.tile([P, F], bf16, name=f"x{i}")
                nc.gpsimd.dma_start(xi[:], X_d[i * P:(i + 1) * P, :])
                x.append(xi)

            # WT/bias/ident8 are only consumed after the AllReduce; they
            # load later on the sync queue (below) to keep the pre-trigger
            # window's DMA bandwidth for H and X.

            # dv chain, phase-batched so each engine runs back-to-back
            # instead of ping-ponging: all rowsums first (split across DVE
            # and ACT; the ACT trash-copies would otherwise clog the sqrt
            # queue), then recips (DVE), sqrts (ACT), Xs muls (DVE).
            trash = sp.tile([P, E], bf16, name="trash")
            dvs = []
            for i in range(NT):
                dv = sp.tile([P, 1], f32, name=f"dv{i}")
                if i % 2 == 0:
                    nc.vector.tensor_reduce(dv[:], h[i][:],
                                            mybir.AxisListType.X,
                                            mybir.AluOpType.add)
                else:
                    nc.scalar.activation(trash[:], h[i][:],
                                         mybir.ActivationFunctionType.Copy,
                                         accum_out=dv[:])
                dvs.append(dv)
            dvrs = []
            for i in range(NT):
                dvr = sp.tile([P, 1], f32, name=f"dvr{i}")
                nc.vector.reciprocal(dvr[:], dvs[i][:])
                dvrs.append(dvr)
            dvis = []
            for i in range(NT):
                dvi = sp.tile([P, 1], f32, name=f"dvis{i}")
                nc.scalar.sqrt(dvi[:], dvrs[i][:])
                dvis.append(dvi)
            xs = []
            for i in range(NT):
                xsi = xp.tile([P, F], bf16, name=f"xs{i}")
                nc.vector.tensor_scalar_mul(xsi[:], x[i][:], dvis[i][:])
                xs.append(xsi)

            # ---- collective bounce buffers: [M^T | de] in bf16 ----
            cc_in = dramp.tile([F + 1, E], bf16, name="cc_in")
            cc_out = dramp.tile([F + 1, E], bf16, name="cc_out",
                                addr_space="Shared")

            # ---- GEMM1 + de, e-half outer. Per half: de partial via
            # ones-stationary matmul (needs only H), then M^T via Xs
            # stationaries; cc_in chunks stream out as they finish.
            for eh in range(E // EH):
                esl = slice(eh * EH, (eh + 1) * EH)

                de_ps = ps_de.tile([1, EH], f32, name="de_ps")
                for i in range(NT):
                    nc.tensor.matmul(
                        de_ps[:], ones[:], h[i][:, esl],
                        start=(i == 0), stop=(i == NT - 1),
                    )
                de_sb = mtoutp.tile([1, EH], bf16, name="de_sb")
                nc.vector.tensor_copy(de_sb[:], de_ps[:])
                nc.sync.dma_start(cc_in[F:F + 1, esl], de_sb[:])

                mt_ps = [ps_mt.tile([P, EH], f32, name=f"mt_ps{jf}")
                         for jf in range(FI)]
                for i in range(NT):
                    for jf in range(FI):
                        nc.tensor.matmul(
                            mt_ps[jf][:],
                            xs[i][:, jf * P:(jf + 1) * P],
                            h[i][:, esl],
                            start=(i == 0), stop=(i == NT - 1),
                        )
                # PSUM -> bf16 casts split across DVE and ACT so the two
                # chunks drain in parallel.
                mt_sb0 = mtoutp.tile([P, EH], bf16, name="mt_sb0")
                nc.vector.tensor_copy(mt_sb0[:], mt_ps[0][:])
                nc.sync.dma_start(cc_in[0:P, esl], mt_sb0[:])
                mt_sb1 = mtoutp.tile([P, EH], bf16, name="mt_sb1")
                nc.scalar.copy(mt_sb1[:], mt_ps[1][:])
                nc.sync.dma_start(cc_in[P:2 * P, esl], mt_sb1[:])

            # ---- H^T tiles: placed on the SYNC queue after the cc_in
            # writes, so their 2 MB transfer happens inside the collective
            # window instead of competing with the H/X loads that gate the
            # trigger. Only GEMM2 consumes them.
            wt = []
            for c in range(FI):
                wtc = constp.tile([P, F], bf16, name=f"wt{c}")
                nc.sync.dma_start(wtc[:], WT_d[c * P:(c + 1) * P, :])
                wt.append(wtc)
            bias = constp.tile([P, F], f32)
            nc.sync.dma_start(bias[:], B_d[:, :])
            id8 = constp.tile([ET, ET], bf16)
            nc.sync.dma_start(id8[:], ID8_d[:, :])
            ht = []
            for j in range(ET):
                htj = htp.tile([P, NL], bf16, name=f"ht{j}")
                nc.sync.dma_start(htj[:], HT_d[j * P:(j + 1) * P, :])
                ht.append(htj)

            # ---- AllReduce of [M^T | de] over all 8 cores (bf16) ----
            nc.gpsimd.collective_compute(
                "AllReduce",
                mybir.AluOpType.add,
                replica_groups=[list(range(NC_COUNT))],
                ins=[cc_in[:].opt()],
                outs=[cc_out[:].opt()],
            )

            # ---- read back: M'^T as ONE fused DMA on the sync queue
            # (gates GEMM-W matmuls); the de row in parallel on the scalar
            # queue as [8, 128] + PE-transpose to [128, 8] (only gates the
            # mw scale step).
            mtin = mwp.tile([P, FI * E], bf16, name="mtin")
            # Both c chunks' first e-tile pulled ahead (32 KB each) so
            # GEMM-W j=0 isn't gated by the full 512 KB readback.
            nc.sync.dma_start(mtin[:, 0:P], cc_out[0:P, 0:P])
            nc.sync.dma_start(mtin[:, E:E + P], cc_out[P:2 * P, 0:P])
            nc.sync.dma_start(mtin[:, P:E], cc_out[0:P, P:E])
            nc.sync.dma_start(mtin[:, E + P:2 * E],
                              cc_out[P:2 * P, P:E])
            de8 = sp.tile([ET, P], bf16)
            nc.scalar.dma_start(
                de8[:],
                cc_out[F:F + 1, :].rearrange("o (c p) -> (o c) p", p=P))
            de_t = ps_de.tile([P, ET], bf16, name="de_t")
            nc.tensor.transpose(de_t[:], de8[:], id8[:])
            de_inv = sp.tile([P, ET], f32)
            nc.vector.reciprocal(de_inv[:], de_t[:])

            # ---- GEMM-W: Mw[e, fo] = (M' @ W^T) * de^-1 ----
            mw = []
            for j in range(ET):
                mw_ps = ps_w.tile([P, F], f32, name="mw_ps")
                for c in range(FI):
                    nc.tensor.matmul(
                        mw_ps[:],
                        mtin[:, c * E + j * P:c * E + (j + 1) * P],
                        wt[c][:],
                        start=(c == 0), stop=(c == FI - 1),
                    )
                mwj = mwp.tile([P, F], bf16, name=f"mw{j}")
                nc.vector.tensor_scalar_mul(mwj[:], mw_ps[:],
                                            de_inv[:, j:j + 1])
                mw.append(mwj)

            # ---- GEMM2: out[n, fo] = (sum_e H^T[e,n] Mw[e,fo]) * dv^-1/2
            #      + b ----
            for jn in range(NT):
                o_ps = ps_o.tile([P, F], f32, name="o_ps")
                for j in range(ET):
                    nc.tensor.matmul(
                        o_ps[:],
                        ht[j][:, jn * P:(jn + 1) * P],
                        mw[j][:],
                        start=(j == 0), stop=(j == ET - 1),
                    )
                ot = outp.tile([P, F], f32, name="ot")
                nc.vector.scalar_tensor_tensor(
                    ot[:], o_ps[:], dvis[jn][:], bias[:],
                    op0=mybir.AluOpType.mult, op1=mybir.AluOpType.add)
                nc.sync.dma_start(out_d[jn * P:(jn + 1) * P, :], ot[:])

    nc.compile()
    return nc


def _get_nc():
    if "nc" not in _cache:
        _cache["nc"] = _build()
    return _cache["nc"]


def kernel(X, H, W, b):
    from concourse import bass_utils

    nc = _get_nc()

    bf16 = ml_dtypes.bfloat16
    X = np.asarray(X, dtype=np.float32)
    H = np.asarray(H, dtype=np.float32)
    W = np.asarray(W, dtype=np.float32)
    b = np.asarray(b, dtype=np.float32)

    WT = np.ascontiguousarray(W.T).astype(bf16)
    bias = np.ascontiguousarray(np.tile(b[None, :], (P, 1)))
    ones_col = np.ones((P, 1), dtype=bf16)
    ident8 = np.eye(ET, dtype=bf16)

    in_maps = []
    for c in range(NC_COUNT):
        sl = slice(c * NL, (c + 1) * NL)
        Hc = np.ascontiguousarray(H[sl])
        in_maps.append({
            "X": np.ascontiguousarray(X[sl]).astype(bf16),
            "H": Hc.astype(bf16),
            "HT": np.ascontiguousarray(Hc.T).astype(bf16),
            "WT": WT,
            "bias": bias,
            "ones": ones_col,
            "ident8": ident8,
        })

    res = bass_utils.run_bass_kernel_spmd(
        nc, in_maps, core_ids=list(range(NC_COUNT)),
        trace=bool(int(os.environ.get("HGNN_TRACE", "0"))),
    )
    _cache["last_result"] = res
    out = np.concatenate([res.results[c]["out"] for c in range(NC_COUNT)],
                         axis=0)
    return out
